# revision 1
# baseline (speedup 1.0000x reference)
"""Bidirectional Mamba layer on 8 Trainium2 NeuronCores (Bass/Tile).

Sharding: 8 cores = 2 directions x 4 batch samples. Each core runs the full
Mamba block for its (direction, sample) pair; a pairwise AllGather combines
the two directions, and every core (redundantly, SPMD-symmetric) applies the
final residual + LayerNorm.

Layout convention on device: channels on partitions, time on the free axis.
The selective scan runs as DVE tensor_tensor_scan (one recurrence per
(channel, state) partition row, time along free).
"""

import ml_dtypes
import numpy as np

import concourse.bass as bass
import concourse.bacc as bacc
import concourse.tile as tile
from concourse import mybir
from concourse.bass_utils import run_bass_kernel_spmd

# ---- problem shapes (hardcoded per contract) ----
B_SZ, L, DM = 4, 2048, 512
D_CONV, DS, DR = 4, 16, 32
DI = 1024                  # d_inner
E2 = 2 * DI                # in_proj rows
NG = DI // 128             # 8 channel blocks
TC = 512                   # time chunk
NCH = L // TC              # 4 chunks
NTT = L // 128             # 16 time tiles of 128
EPS = 1e-5
F32 = mybir.dt.float32
F32R = mybir.dt.float32r
BF16 = mybir.dt.bfloat16
AF = mybir.ActivationFunctionType
OP = mybir.AluOpType

N_CORES = 8

# CoreSim does not implement Silu; tests flip this to use sigmoid+mult
USE_SILU = True
REPLICA_GROUPS = [[0, 4], [1, 5], [2, 6], [3, 7]]

# y-accumulation PSUM wave: process channel blocks in waves of WAVE so the
# per-block accumulators fit in PSUM alongside the matmul pipeline banks.
WAVE = 4


def r(ap):
    """View an fp32 AP as float32r (used for DMA-side dtype reconciliation)."""
    return ap.bitcast(F32R)


def _silu(nc, pool, out_tile, psum, bias):
    """out = silu(psum + bias); Silu on HW, sigmoid+mult fallback for CoreSim."""
    if USE_SILU:
        nc.scalar.activation(out_tile, psum, AF.Silu, bias=bias)
    else:
        zb = pool.tile(list(out_tile.shape), F32, tag="_silu_zb", name="zb",
                       bufs=1)
        nc.scalar.activation(zb, psum, AF.Identity, bias=bias)
        nc.scalar.activation(out_tile, psum, AF.Sigmoid, bias=bias)
        nc.vector.tensor_mul(out_tile, zb, out_tile)


def build_module():
    nc = bacc.Bacc(
        "TRN2", target_bir_lowering=False, debug=False, num_devices=N_CORES
    )

    # ---------------- I/O ----------------
    x_d = nc.dram_tensor("x_d", [L, DM], F32, kind="ExternalInput")
    x_nat = nc.dram_tensor("x_nat", [L, DM], F32, kind="ExternalInput")
    w_in = nc.dram_tensor("w_in", [DM, E2], F32R, kind="ExternalInput")
    convd = nc.dram_tensor("convd", [D_CONV, NG, 128, 128], F32R, kind="ExternalInput")
    convb = nc.dram_tensor("convb", [NG, 128], F32, kind="ExternalInput")
    silub = nc.dram_tensor("silub", [NG, 128], F32, kind="ExternalInput")
    w_xp = nc.dram_tensor("w_xp", [DI, DR + 2 * DS], F32R, kind="ExternalInput")
    w_dt = nc.dram_tensor("w_dt", [DR, DI], F32R, kind="ExternalInput")
    dtb = nc.dram_tensor("dtb", [NG, 128], F32, kind="ExternalInput")
    a_sc = nc.dram_tensor("a_sc", [NG, 128, DS], F32, kind="ExternalInput")
    d_vec = nc.dram_tensor("d_vec", [NG, 128], F32, kind="ExternalInput")
    w_out = nc.dram_tensor("w_out", [DI, DM], F32R, kind="ExternalInput")
    eye = nc.dram_tensor("eye", [128, 128], F32, kind="ExternalInput")
    eyen = nc.dram_tensor("eyen", [128, 128], BF16, kind="ExternalInput")
    eyej = nc.dram_tensor("eyej", [128, 128], F32R, kind="ExternalInput")
    ln2w = nc.dram_tensor("ln2w", [1, DM], F32, kind="ExternalInput")
    ln2b = nc.dram_tensor("ln2b", [1, DM], F32, kind="ExternalInput")
    out = nc.dram_tensor("out", [L, DM], F32, kind="ExternalOutput")

    with tile.TileContext(nc) as tc:
        build_program(
            tc, x_d, x_nat, w_in, convd, convb, silub, w_xp, w_dt, dtb,
            a_sc, d_vec, w_out, eye, eyen, eyej, ln2w, ln2b, out,
        )
    nc.compile()
    return nc


def build_program(tc, x_d, x_nat, w_in, convd, convb, silub, w_xp, w_dt, dtb,
                  a_sc, d_vec, w_out, eye, eyen, eyej, ln2w, ln2b, out):
    nc = tc.nc
    from contextlib import ExitStack

    ctx = ExitStack()
    with ctx:
        wp = ctx.enter_context(tc.tile_pool(name="wp", bufs=1))
        lnp = ctx.enter_context(tc.tile_pool(name="lnp", bufs=2))
        chp = ctx.enter_context(tc.tile_pool(name="chp", bufs=1))
        snp = ctx.enter_context(tc.tile_pool(name="snp", bufs=2))
        repp = ctx.enter_context(tc.tile_pool(name="repp", bufs=2))
        pp = ctx.enter_context(tc.tile_pool(name="pp", space="PSUM", bufs=2))
        ppy = ctx.enter_context(tc.tile_pool(name="ppy", space="PSUM", bufs=1))
        dramp = ctx.enter_context(tc.tile_pool(name="dramp", bufs=2, space="DRAM"))

        # ---------------- load weights ----------------
        w_in_sb = wp.tile([128, DM // 128, E2], F32R)
        for k in range(DM // 128):
            nc.sync.dma_start(w_in_sb[:, k, :], w_in[k * 128:(k + 1) * 128, :])
        convd_sb = wp.tile([128, D_CONV, NG, 128], F32R)
        for k in range(D_CONV):
            for g in range(NG):
                nc.sync.dma_start(convd_sb[:, k, g, :], convd[k, g])
        w_xp_sb = wp.tile([128, NG, DR + 2 * DS], F32R)
        for g in range(NG):
            nc.sync.dma_start(w_xp_sb[:, g, :], w_xp[g * 128:(g + 1) * 128, :])
        w_dt_sb = wp.tile([DR, DI], F32R)
        nc.sync.dma_start(w_dt_sb, w_dt[:, :])
        w_out_sb = wp.tile([128, NG, DM], F32R)
        for g in range(NG):
            nc.sync.dma_start(w_out_sb[:, g, :], w_out[g * 128:(g + 1) * 128, :])
        a_sb = wp.tile([128, NG, DS], F32)
        for g in range(NG):
            nc.sync.dma_start(a_sb[:, g, :], a_sc[g])
        eye_sb = wp.tile([128, 128], F32)
        nc.sync.dma_start(eye_sb, eye[:, :])
        eyen_sb = wp.tile([128, 128], BF16)
        nc.sync.dma_start(eyen_sb, eyen[:, :])
        eyej_sb = wp.tile([128, 128], F32R)
        nc.sync.dma_start(eyej_sb, eyej[:, :])

        def col_load(name, src):
            t = wp.tile([128, NG], F32, name=name)
            for g in range(NG):
                nc.sync.dma_start(t[:, g:g + 1], src[g:g + 1, :].rearrange("a b -> b a"))
            return t

        convb_sb = col_load("convb_sb", convb)
        silub_sb = col_load("silub_sb", silub)
        dtb_sb = col_load("dtb_sb", dtb)
        d_sb = col_load("d_sb", d_vec)

        ln2w_sb = wp.tile([128, DM], F32)
        nc.sync.dma_start(ln2w_sb, ln2w[0:1, :].to_broadcast([128, DM]))
        ln2b_sb = wp.tile([128, DM], F32)
        nc.sync.dma_start(ln2b_sb, ln2b[0:1, :].to_broadcast([128, DM]))

        eps_sb = wp.tile([128, 1], F32)
        nc.vector.memset(eps_sb, EPS)
        # per-(g,n) scan carry, column n of block g; zero before chunk 0
        hlast = wp.tile([128, NG, DS], BF16)
        nc.vector.memset(hlast, 0.0)

        ypart = dramp.tile([L, DM], F32, bufs=1)
        gbuf = dramp.tile([2, L, DM], F32, bufs=1)

        # conv staging with 3-column causal halo; persistent across chunks so
        # the halo copy reads the previous chunk's columns from the same tile
        xz = [chp.tile([128, TC + 3], F32R, name=f"xz{g}") for g in range(NG)]

        # ---------------- per-chunk pipeline ----------------
        for c in range(NCH):
            t0 = c * TC

            # -- A: LayerNorm 1 (affine folded into weights) + transpose --
            xn_t = chp.tile([128, DM // 128, TC], F32R, tag="xn_t")
            for i in range(TC // 128):
                xt = lnp.tile([128, DM], F32, tag="xt")
                nc.sync.dma_start(xt, x_d[t0 + i * 128: t0 + (i + 1) * 128, :])
                st = lnp.tile([128, 6], F32, tag="st")
                nc.vector.bn_stats(st, xt)
                mv = lnp.tile([128, 2], F32, tag="mv")
                nc.vector.bn_aggr(mv, st)
                nc.scalar.activation(mv[:, 1:2], mv[:, 1:2], AF.Ln,
                                     bias=eps_sb[:, 0:1])
                nc.scalar.activation(mv[:, 1:2], mv[:, 1:2], AF.Exp, scale=-0.5)
                nc.vector.tensor_scalar(out=xt, in0=xt, scalar1=mv[:, 0:1],
                                        scalar2=mv[:, 1:2],
                                        op0=OP.subtract, op1=OP.mult)
                for kb in range(DM // 128):
                    ps_t = pp.tile([128, 128], F32, tag="trps", bufs=1)
                    nc.tensor.transpose(ps_t, xt[:, kb * 128:(kb + 1) * 128],
                                        eye_sb)
                    nc.any.tensor_copy(xn_t[:, kb, i * 128:(i + 1) * 128], ps_t)

            # -- B: in_proj (xc half) into conv staging with halo --
            for g in range(NG):
                if c == 0:
                    nc.vector.memset(xz[g][:, 0:3].bitcast(F32), 0.0)
                else:
                    nc.vector.tensor_copy(xz[g][:, 0:3], xz[g][:, TC:TC + 3])
                pz = pp.tile([128, TC], F32, tag="ps")
                for k in range(DM // 128):
                    nc.tensor.matmul(pz, w_in_sb[:, k, g * 128:(g + 1) * 128],
                                     xn_t[:, k, :],
                                     start=(k == 0), stop=(k == DM // 128 - 1))
                nc.vector.tensor_copy(xz[g][:, 3:TC + 3], pz)

            # -- B2: in_proj (z half) + SiLU gate --
            sg = [chp.tile([128, TC], BF16, tag=f"sg{g}", name=f"sg{g}") for g in range(NG)]
            for g in range(NG):
                pz = pp.tile([128, TC], F32, tag="ps")
                for k in range(DM // 128):
                    nc.tensor.matmul(
                        pz, w_in_sb[:, k, DI + g * 128: DI + (g + 1) * 128],
                        xn_t[:, k, :],
                        start=(k == 0), stop=(k == DM // 128 - 1))
                _silu(nc, snp, sg[g], pz, silub_sb[:, g:g + 1])

            # -- C: depthwise causal conv + SiLU --
            xc = [chp.tile([128, TC], F32R, tag=f"xc{g}", name=f"xc{g}") for g in range(NG)]
            for g in range(NG):
                pc = pp.tile([128, TC], F32, tag="ps")
                for k in range(D_CONV):
                    nc.tensor.matmul(pc, convd_sb[:, k, g, :],
                                     xz[g][:, k:k + TC],
                                     start=(k == 0), stop=(k == D_CONV - 1))
                _silu(nc, snp, xc[g], pc, convb_sb[:, g:g + 1])

            # -- D: x_proj -> (dt_r, B, C) --
            pxd = pp.tile([64, TC], F32, tag="pxd", bufs=1)
            for g in range(NG):
                nc.tensor.matmul(pxd, w_xp_sb[:, g, :], xc[g],
                                 start=(g == 0), stop=(g == NG - 1))
            xdbl = chp.tile([64, TC], F32R, tag="xdbl", bufs=2)
            nc.vector.tensor_copy(xdbl, pxd)
            bc_d = dramp.tile([2 * DS, TC], BF16, tag="bc_d", bufs=2)
            bc_bf = chp.tile([2 * DS, TC], BF16, tag="bc_bf", bufs=2)
            nc.vector.tensor_copy(bc_bf, xdbl[DR:DR + 2 * DS, :])
            nc.sync.dma_start(bc_d, bc_bf)

            # -- D2: dt = softplus(dt_proj @ dt_r + bias); P = dt * xc --
            dt_c = [chp.tile([128, TC], F32, tag=f"dt{g}", name=f"dtc{g}") for g in range(NG)]
            p_c = [chp.tile([128, TC], BF16, tag=f"p{g}", name=f"pc{g}") for g in range(NG)]
            for g in range(NG):
                pdt = pp.tile([128, TC], F32, tag="ps")
                nc.tensor.matmul(pdt, w_dt_sb[:, g * 128:(g + 1) * 128],
                                 xdbl[0:DR, :], start=True, stop=True)
                nc.scalar.activation(dt_c[g], pdt, AF.Exp,
                                     bias=dtb_sb[:, g:g + 1])
                nc.scalar.activation(dt_c[g], dt_c[g], AF.Ln, bias=1.0)
                nc.vector.tensor_mul(p_c[g], dt_c[g], xc[g])

            # -- E/F: selective scan, wave over channel blocks --
            yf = [chp.tile([128, TC], F32R, tag=f"xc{g}", name=f"yf{g}") for g in range(NG)]
            for w0 in range(0, NG, WAVE):
                py = [ppy.tile([128, TC], F32, tag=f"py{j}", bufs=1, name=f"py{j}")
                      for j in range(WAVE)]
                # seed with the D-term: xc * D
                for j in range(WAVE):
                    g = w0 + j
                    yd = snp.tile([128, TC], F32, tag="yd")
                    nc.vector.tensor_scalar_mul(yd, xc[g], d_sb[:, g:g + 1])
                    nc.tensor.matmul(py[j], eye_sb, yd,
                                     start=True, stop=False)
                for n in range(DS):
                    brep = repp.tile([128, TC], BF16, tag="brep")
                    nc.sync.dma_start(brep, bc_d[n:n + 1, :].to_broadcast([128, TC]))
                    crep = repp.tile([128, TC], BF16, tag="crep")
                    nc.sync.dma_start(crep,
                                      bc_d[DS + n:DS + n + 1, :].to_broadcast([128, TC]))
                    for j in range(WAVE):
                        g = w0 + j
                        da = snp.tile([128, TC], F32, tag="da")
                        nc.scalar.activation(da, dt_c[g], AF.Exp,
                                             scale=a_sb[:, g, n:n + 1])
                        bt = snp.tile([128, TC], BF16, tag="bt")
                        nc.vector.tensor_mul(bt, p_c[g], brep)
                        h = snp.tile([128, TC], BF16, tag="h")
                        nc.vector.tensor_tensor_scan(
                            h, da, bt, initial=hlast[:, g, n:n + 1],
                            op0=OP.mult, op1=OP.add)
                        nc.sync.dma_start(hlast[:, g, n:n + 1],
                                              h[:, TC - 1:TC])
                        yn = snp.tile([128, TC], BF16, tag="yn")
                        nc.vector.tensor_mul(yn, h, crep)
                        nc.tensor.matmul(py[j], eyen_sb, yn,
                                         start=False, stop=(n == DS - 1))
                # -- G: gate with silu(z) --
                for j in range(WAVE):
                    g = w0 + j
                    nc.vector.tensor_mul(yf[g], py[j], sg[g])

            # -- H: out_proj, emitted directly in [t, dm] layout --
            for tb in range(TC // 128):
                po = pp.tile([128, DM], F32, tag="ps")
                for g in range(NG):
                    nc.tensor.matmul(po, yf[g][:, tb * 128:(tb + 1) * 128],
                                     w_out_sb[:, g, :],
                                     start=(g == 0), stop=(g == NG - 1))
                ot = lnp.tile([128, DM], F32, tag="ot")
                nc.vector.tensor_copy(ot, po)
                nc.sync.dma_start(
                    ypart[t0 + tb * 128: t0 + (tb + 1) * 128, :], ot)

        # ---------------- combine directions + final LayerNorm ----------------
        nc.gpsimd.collective_compute(
            "AllGather", OP.bypass, replica_groups=REPLICA_GROUPS,
            ins=[ypart.opt()], outs=[gbuf.opt()],
        )
        for i in range(NTT):
            s0 = lnp.tile([128, DM], F32, tag="xt")
            nc.sync.dma_start(s0, gbuf[0, i * 128:(i + 1) * 128, :])
            # direction-1 partial is in flipped time order: load the
            # mirrored block forward, then reverse rows via the exchange
            # matrix on the tensor engine (DMA cannot step partitions
            # backwards)
            s1 = lnp.tile([128, DM], F32R, tag="ot")
            nc.sync.dma_start(
                s1, gbuf[1, L - (i + 1) * 128: L - i * 128, :].bitcast(F32R))
            pj = pp.tile([128, DM], F32, tag="ps")
            nc.tensor.matmul(pj, eyej_sb, s1, start=True, stop=True)
            xt2 = lnp.tile([128, DM], F32, tag="xt2")
            nc.sync.dma_start(xt2, x_nat[i * 128:(i + 1) * 128, :])
            nc.vector.tensor_add(s0, s0, pj)
            nc.vector.tensor_add(s0, s0, xt2)
            st = lnp.tile([128, 6], F32, tag="st")
            nc.vector.bn_stats(st, s0)
            mv = lnp.tile([128, 2], F32, tag="mv")
            nc.vector.bn_aggr(mv, st)
            nc.scalar.activation(mv[:, 1:2], mv[:, 1:2], AF.Ln,
                                 bias=eps_sb[:, 0:1])
            nc.scalar.activation(mv[:, 1:2], mv[:, 1:2], AF.Exp, scale=-0.5)
            nc.vector.tensor_scalar(out=s0, in0=s0, scalar1=mv[:, 0:1],
                                    scalar2=mv[:, 1:2],
                                    op0=OP.subtract, op1=OP.mult)
            nc.vector.tensor_mul(s0, s0, ln2w_sb)
            nc.vector.tensor_add(s0, s0, ln2b_sb)
            nc.sync.dma_start(out[i * 128:(i + 1) * 128, :], s0)


# ---------------- host side ----------------

def make_core_inputs(inputs):
    """Build the 8 per-core input dicts from the full problem inputs."""
    x = np.ascontiguousarray(inputs["x"], dtype=np.float32)        # [B, L, DM]
    ln1_w = inputs["ln1_w"].astype(np.float32)
    ln1_b = inputs["ln1_b"].astype(np.float32)
    in_w = inputs["in_proj_w"].astype(np.float32)                  # [2, 2DI, DM]
    conv_w = inputs["conv_w"].astype(np.float32)                   # [2, DI, 4]
    conv_b = inputs["conv_b"].astype(np.float32)                   # [2, DI]
    xp_w = inputs["x_proj_w"].astype(np.float32)                   # [2, 64, DI]
    dt_w = inputs["dt_proj_w"].astype(np.float32)                  # [2, DI, DR]
    dt_b = inputs["dt_proj_b"].astype(np.float32)                  # [2, DI]
    a_log = inputs["A_log"].astype(np.float32)                     # [2, DI, DS]
    d_par = inputs["D_param"].astype(np.float32)                   # [2, DI]
    out_w = inputs["out_proj_w"].astype(np.float32)                # [2, DM, DI]
    ln2_w = inputs["ln2_w"].astype(np.float32)
    ln2_b = inputs["ln2_b"].astype(np.float32)

    eye = np.eye(128, dtype=np.float32)
    per_dir = []
    for d in range(2):
        w = in_w[d]                                   # [2DI, DM]
        w_in_T = np.ascontiguousarray((w * ln1_w[None, :]).T)      # [DM, 2DI]
        v = w @ ln1_b                                  # [2DI]
        csum = conv_w[d].sum(axis=1)                   # [DI]
        convb_adj = conv_b[d] + csum * v[:DI]
        silub_adj = v[DI:]
        convd = np.zeros((D_CONV, NG, 128, 128), np.float32)
        for k in range(D_CONV):
            for g in range(NG):
                np.fill_diagonal(convd[k, g], conv_w[d, g * 128:(g + 1) * 128, k])
        a_neg = -np.exp(a_log[d])                      # [DI, DS]
        per_dir.append(dict(
            w_in=w_in_T,
            convd=convd,
            convb=convb_adj.reshape(NG, 128),
            silub=silub_adj.reshape(NG, 128),
            w_xp=np.ascontiguousarray(xp_w[d].T),      # [DI, 64]
            w_dt=np.ascontiguousarray(dt_w[d].T),      # [DR, DI]
            dtb=dt_b[d].reshape(NG, 128),
            a_sc=np.ascontiguousarray(a_neg.reshape(NG, 128, DS)),
            d_vec=d_par[d].reshape(NG, 128),
            w_out=np.ascontiguousarray(out_w[d].T),    # [DI, DM]
        ))

    in_maps = []
    for core in range(N_CORES):
        d, b = core // 4, core % 4
        xb = x[b]
        m = dict(per_dir[d])
        m["x_d"] = xb if d == 0 else np.ascontiguousarray(xb[::-1])
        m["x_nat"] = xb
        m["eye"] = eye
        m["eyen"] = eye.astype(ml_dtypes.bfloat16)
        m["eyej"] = eye[::-1].copy()
        m["ln2w"] = ln2_w.reshape(1, DM)
        m["ln2b"] = ln2_b.reshape(1, DM)
        in_maps.append(m)
    return in_maps


_NC = None


def _get_module():
    global _NC
    if _NC is None:
        _NC = build_module()
    return _NC


def kernel(**inputs) -> np.ndarray:
    nc = _get_module()
    in_maps = make_core_inputs(inputs)
    res = run_bass_kernel_spmd(nc, in_maps, core_ids=list(range(N_CORES)))
    outs = [res.results[b]["out"] for b in range(B_SZ)]
    return np.stack(outs, axis=0)


if __name__ == "__main__":
    nc = build_module()
    print("module built ok")



# revision 6
# speedup vs baseline: 1.0351x; 1.0351x over previous
"""Bidirectional Mamba layer on 8 Trainium2 NeuronCores (Bass/Tile).

Sharding: 8 cores = 2 directions x 4 batch samples. Each core runs the full
Mamba block for its (direction, sample) pair; a pairwise AllGather combines
the two directions, and every core (redundantly, SPMD-symmetric) applies the
final residual + LayerNorm.

Layout: channels on partitions, time on the free axis, full L=2048 per op
(monolith — no time chunking, so the selective scan needs no state carry).
All matmuls run bf16 (single PE pass); elementwise work is bf16 for the DVE
2x mode. The per-(group, state) inner loop is:
  Act:  da = exp(dt * A[g,n])          (scalar engine)
  DVE/GPS: bt = p * B_n                (broadcast multiply)
  DVE:  h = tensor_tensor_scan(da, bt) (the irreducible 2 cyc/elem scan)
  DVE:  yn = h * C_n
  PE:   py += I @ yn                   (PSUM accumulation over states)
"""

import ml_dtypes
import numpy as np

import concourse.bass as bass
import concourse.bacc as bacc
import concourse.tile as tile
from concourse import mybir
from concourse.bass_utils import run_bass_kernel_spmd

# ---- problem shapes (hardcoded per contract) ----
B_SZ, L, DM = 4, 2048, 512
D_CONV, DS, DR = 4, 16, 32
DI = 1024                  # d_inner
E2 = 2 * DI                # in_proj rows
NG = DI // 128             # 8 channel blocks
NTT = L // 128             # 16 time tiles of 128
NSL = L // 512             # 4 psum-width slices
EPS = 1e-5
F32 = mybir.dt.float32
BF16 = mybir.dt.bfloat16
AF = mybir.ActivationFunctionType
OP = mybir.AluOpType

N_CORES = 8

# CoreSim does not implement Silu; tests flip this to use sigmoid+mult
USE_SILU = True
REPLICA_GROUPS = [[0, 4], [1, 5], [2, 6], [3, 7]]

# states whose bt-multiply runs on GPSIMD instead of DVE
GPS_NS = (1, 3, 5, 7, 9, 11, 13, 15)


def _silu(nc, pool, out_tile, psum, bias=None):
    """out = silu(psum [+ bias]); Silu on HW, sigmoid+mult fallback for sim."""
    kw = {} if bias is None else {"bias": bias}
    if USE_SILU:
        nc.scalar.activation(out_tile, psum, AF.Silu, **kw)
    else:
        zb = pool.tile(list(out_tile.shape), F32, tag="_silu_zb", name="zb",
                       bufs=1)
        nc.scalar.activation(zb, psum, AF.Identity, **kw)
        nc.scalar.activation(out_tile, psum, AF.Sigmoid, **kw)
        nc.vector.tensor_mul(out_tile, zb, out_tile)


def build_module():
    nc = bacc.Bacc(
        "TRN2", target_bir_lowering=False, debug=False, num_devices=N_CORES
    )

    # ---------------- I/O ----------------
    x_d = nc.dram_tensor("x_d", [L, DM], F32, kind="ExternalInput")
    x_nat = nc.dram_tensor("x_nat", [L, DM], BF16, kind="ExternalInput")
    w_in = nc.dram_tensor("w_in", [DM, E2], BF16, kind="ExternalInput")
    convd = nc.dram_tensor("convd", [D_CONV, NG, 128, 128], BF16, kind="ExternalInput")
    convb = nc.dram_tensor("convb", [NG, 128], F32, kind="ExternalInput")
    silub = nc.dram_tensor("silub", [NG, 128], F32, kind="ExternalInput")
    w_xp = nc.dram_tensor("w_xp", [DI, DR + 2 * DS], BF16, kind="ExternalInput")
    w_dt = nc.dram_tensor("w_dt", [DR, DI], BF16, kind="ExternalInput")
    dtb = nc.dram_tensor("dtb", [NG, 128], F32, kind="ExternalInput")
    a_sc = nc.dram_tensor("a_sc", [NG, 128, DS], F32, kind="ExternalInput")
    d_vec = nc.dram_tensor("d_vec", [NG, 128], F32, kind="ExternalInput")
    w_out = nc.dram_tensor("w_out", [DI, DM], BF16, kind="ExternalInput")
    eye = nc.dram_tensor("eye", [128, 128], F32, kind="ExternalInput")
    eyen = nc.dram_tensor("eyen", [128, 128], BF16, kind="ExternalInput")
    eyej = nc.dram_tensor("eyej", [128, 128], BF16, kind="ExternalInput")
    ln2w = nc.dram_tensor("ln2w", [1, DM], F32, kind="ExternalInput")
    ln2b = nc.dram_tensor("ln2b", [1, DM], F32, kind="ExternalInput")
    out = nc.dram_tensor("out", [L, DM], F32, kind="ExternalOutput")

    with tile.TileContext(nc) as tc:
        build_program(
            tc, x_d, x_nat, w_in, convd, convb, silub, w_xp, w_dt, dtb,
            a_sc, d_vec, w_out, eye, eyen, eyej, ln2w, ln2b, out,
        )
    nc.compile()
    return nc


def build_program(tc, x_d, x_nat, w_in, convd, convb, silub, w_xp, w_dt, dtb,
                  a_sc, d_vec, w_out, eye, eyen, eyej, ln2w, ln2b, out):
    nc = tc.nc
    from contextlib import ExitStack

    ctx = ExitStack()
    with ctx:
        wp = ctx.enter_context(tc.tile_pool(name="wp", bufs=1))
        lnp = ctx.enter_context(tc.tile_pool(name="lnp", bufs=2))
        big = ctx.enter_context(tc.tile_pool(name="big", bufs=1))
        snp = ctx.enter_context(tc.tile_pool(name="snp", bufs=3))
        repp = ctx.enter_context(tc.tile_pool(name="repp", bufs=3))
        pp = ctx.enter_context(tc.tile_pool(name="pp", space="PSUM", bufs=2))
        ppy = ctx.enter_context(tc.tile_pool(name="ppy", space="PSUM", bufs=1))
        dramp = ctx.enter_context(tc.tile_pool(name="dramp", bufs=2, space="DRAM"))

        # ---------------- load weights ----------------
        w_in_sb = wp.tile([128, DM // 128, E2], BF16)
        for k in range(DM // 128):
            nc.sync.dma_start(w_in_sb[:, k, :], w_in[k * 128:(k + 1) * 128, :])
        convd_sb = wp.tile([128, D_CONV, NG, 128], BF16)
        for k in range(D_CONV):
            for g in range(NG):
                nc.sync.dma_start(convd_sb[:, k, g, :], convd[k, g])
        w_xp_sb = wp.tile([128, NG, DR + 2 * DS], BF16)
        for g in range(NG):
            nc.sync.dma_start(w_xp_sb[:, g, :], w_xp[g * 128:(g + 1) * 128, :])
        w_dt_sb = wp.tile([DR, DI], BF16)
        nc.sync.dma_start(w_dt_sb, w_dt[:, :])
        w_out_sb = wp.tile([128, NG, DM], BF16)
        for g in range(NG):
            nc.sync.dma_start(w_out_sb[:, g, :], w_out[g * 128:(g + 1) * 128, :])
        a_sb = wp.tile([128, NG, DS], F32)
        for g in range(NG):
            nc.sync.dma_start(a_sb[:, g, :], a_sc[g])
        eye_sb = wp.tile([128, 128], F32)
        nc.sync.dma_start(eye_sb, eye[:, :])
        eyen_sb = wp.tile([128, 128], BF16)
        nc.sync.dma_start(eyen_sb, eyen[:, :])
        eyej_sb = wp.tile([128, 128], BF16)
        nc.sync.dma_start(eyej_sb, eyej[:, :])

        def col_load(name, src):
            t = wp.tile([128, NG], F32, name=name)
            for g in range(NG):
                nc.sync.dma_start(t[:, g:g + 1], src[g:g + 1, :].rearrange("a b -> b a"))
            return t

        convb_sb = col_load("convb_sb", convb)
        silub_sb = col_load("silub_sb", silub)
        dtb_sb = col_load("dtb_sb", dtb)
        d_sb = col_load("d_sb", d_vec)

        ln2w_sb = wp.tile([128, DM], F32)
        nc.sync.dma_start(ln2w_sb, ln2w[0:1, :].to_broadcast([128, DM]))
        ln2b_sb = wp.tile([128, DM], F32)
        nc.sync.dma_start(ln2b_sb, ln2b[0:1, :].to_broadcast([128, DM]))

        eps_sb = wp.tile([128, 1], F32)
        nc.vector.memset(eps_sb, EPS)

        ypart = dramp.tile([L, DM], BF16, bufs=1)
        sg_d = dramp.tile([NG, 128, L], BF16, bufs=1)
        gbuf = dramp.tile([2, L, DM], BF16, bufs=1)
        bc_d = dramp.tile([2 * DS, L], BF16, bufs=1)

        # ---------------- phase A: LayerNorm1 + transpose ----------------
        # xn_t: [dm-part, k-block, t] bf16, consumed by in_proj
        xn_t = big.tile([128, DM // 128, L], BF16, name="xn_t")
        for i in range(NTT):
            xt = lnp.tile([128, DM], F32, tag="xt")
            nc.sync.dma_start(xt, x_d[i * 128:(i + 1) * 128, :])
            st = lnp.tile([128, 6], F32, tag="st")
            nc.vector.bn_stats(st, xt)
            mv = lnp.tile([128, 2], F32, tag="mv")
            nc.vector.bn_aggr(mv, st)
            nc.scalar.activation(mv[:, 1:2], mv[:, 1:2], AF.Ln,
                                 bias=eps_sb[:, 0:1])
            nc.scalar.activation(mv[:, 1:2], mv[:, 1:2], AF.Exp, scale=-0.5)
            nc.vector.tensor_scalar(out=xt, in0=xt, scalar1=mv[:, 0:1],
                                    scalar2=mv[:, 1:2],
                                    op0=OP.subtract, op1=OP.mult)
            for kb in range(DM // 128):
                ps_t = pp.tile([128, 512], F32, tag="ps")
                nc.tensor.transpose(ps_t[:, 0:128], xt[:, kb * 128:(kb + 1) * 128],
                                    eye_sb)
                nc.vector.tensor_copy(xn_t[:, kb, i * 128:(i + 1) * 128],
                                      ps_t[:, 0:128])

        # ---------------- phase B: in_proj ----------------
        # xc half into conv staging (3-col zero halo at front)
        xzf = [big.tile([128, L + 3], BF16, name=f"xzf{g}") for g in range(NG)]
        for g in range(NG):
            nc.vector.memset(xzf[g][:, 0:3], 0.0)
            for s in range(NSL):
                pz = pp.tile([128, 512], F32, tag="ps")
                for k in range(DM // 128):
                    nc.tensor.matmul(pz, w_in_sb[:, k, g * 128:(g + 1) * 128],
                                     xn_t[:, k, s * 512:(s + 1) * 512],
                                     start=(k == 0), stop=(k == DM // 128 - 1))
                nc.vector.tensor_copy(xzf[g][:, 3 + s * 512:3 + (s + 1) * 512], pz)

        # z half + SiLU gate -> sg (spilled to DRAM, reloaded at gate)
        for g in range(NG):
            for s in range(NSL):
                pz = pp.tile([128, 512], F32, tag="ps")
                for k in range(DM // 128):
                    nc.tensor.matmul(
                        pz, w_in_sb[:, k, DI + g * 128: DI + (g + 1) * 128],
                        xn_t[:, k, s * 512:(s + 1) * 512],
                        start=(k == 0), stop=(k == DM // 128 - 1))
                sgt = snp.tile([128, 512], BF16, tag="sgt", bufs=2)
                _silu(nc, snp, sgt, pz, silub_sb[:, g:g + 1])
                nc.sync.dma_start(sg_d[g][:, s * 512:(s + 1) * 512], sgt)

        # ---------------- phase C: depthwise causal conv + SiLU ----------
        xc = [big.tile([128, L], BF16, name=f"xc{g}") for g in range(NG)]
        for g in range(NG):
            for s in range(NSL):
                pc = pp.tile([128, 512], F32, tag="ps")
                for k in range(D_CONV):
                    nc.tensor.matmul(pc, convd_sb[:, k, g, :],
                                     xzf[g][:, k + s * 512: k + (s + 1) * 512],
                                     start=(k == 0), stop=(k == D_CONV - 1))
                _silu(nc, snp, xc[g][:, s * 512:(s + 1) * 512], pc,
                      convb_sb[:, g:g + 1])

        # ---------------- phase D: x_proj -> (dt_r, B, C) -----------------
        xdbl = big.tile([64, L], BF16, name="xdbl")
        for s in range(NSL):
            pxd = pp.tile([64, 512], F32, tag="pxd", bufs=1)
            for g in range(NG):
                nc.tensor.matmul(pxd, w_xp_sb[:, g, :],
                                 xc[g][:, s * 512:(s + 1) * 512],
                                 start=(g == 0), stop=(g == NG - 1))
            nc.vector.tensor_copy(xdbl[:, s * 512:(s + 1) * 512], pxd)
        nc.sync.dma_start(bc_d, xdbl[DR:DR + 2 * DS, :])

        # dt = softplus(dt_proj @ dt_r + bias); p = dt * xc
        # dt_c[0..3] reuse xn_t's space, dt_c[4..7] reuse w_in_sb (both dead
        # after in_proj); p_c reuses the conv staging xzf (dead after conv)
        dt_c = [xn_t[:, g, :] if g < 4 else w_in_sb[:, g - 4, :]
                for g in range(NG)]
        p_c = [xzf[g][:, 0:L] for g in range(NG)]
        for g in range(NG):
            for s in range(NSL):
                sl = slice(s * 512, (s + 1) * 512)
                pdt = pp.tile([128, 512], F32, tag="ps")
                nc.tensor.matmul(pdt, w_dt_sb[:, g * 128:(g + 1) * 128],
                                 xdbl[0:DR, sl],
                                 start=True, stop=True)
                nc.scalar.activation(dt_c[g][:, sl], pdt, AF.Exp,
                                     bias=dtb_sb[:, g:g + 1])
                nc.scalar.activation(dt_c[g][:, sl], dt_c[g][:, sl],
                                     AF.Ln, bias=1.0)
            nc.vector.tensor_mul(p_c[g], dt_c[g], xc[g])

        # ---------------- phase E: selective scan, g-outer ----------------
        yf = [None] * NG
        for g in range(NG):
            py = ppy.tile([128, L], F32, tag="py", bufs=1)
            # seed with the D-term: xc * D
            yd = snp.tile([128, L], BF16, tag="yd", bufs=1)
            nc.vector.tensor_scalar_mul(yd, xc[g], d_sb[:, g:g + 1])
            for s in range(NSL):
                nc.tensor.matmul(py[:, s * 512:(s + 1) * 512], eyen_sb,
                                 yd[:, s * 512:(s + 1) * 512],
                                 start=True, stop=False)
            for n in range(DS):
                brep = repp.tile([128, L], BF16, tag="brep", bufs=2)
                nc.sync.dma_start(brep, bc_d[n:n + 1, :].to_broadcast([128, L]))
                crep = repp.tile([128, L], BF16, tag="crep", bufs=2)
                nc.sync.dma_start(crep,
                                  bc_d[DS + n:DS + n + 1, :].to_broadcast([128, L]))
                da = snp.tile([128, L], BF16, tag="da", bufs=2)
                nc.scalar.activation(da, dt_c[g], AF.Exp,
                                     scale=a_sb[:, g, n:n + 1])
                if n in GPS_NS:
                    bt = snp.tile([128, L], BF16, tag="bt", name="btg", bufs=3)
                    nc.gpsimd.tensor_tensor(bt, p_c[g], brep, op=OP.mult)
                else:
                    bt = snp.tile([128, L], BF16, tag="bt", name="btd", bufs=3)
                    nc.vector.tensor_mul(bt, p_c[g], brep)
                h = snp.tile([128, L], BF16, tag="h", bufs=2)
                nc.vector.tensor_tensor_scan(h, da, bt, initial=0.0,
                                             op0=OP.mult, op1=OP.add)
                yn = snp.tile([128, L], BF16, tag="yn", bufs=2)
                nc.vector.tensor_mul(yn, h, crep)
                for s in range(NSL):
                    nc.tensor.matmul(py[:, s * 512:(s + 1) * 512], eyen_sb,
                                     yn[:, s * 512:(s + 1) * 512],
                                     start=False, stop=(n == DS - 1))
            # gate with silu(z); write into xc[g]'s tile (xc dead now)
            sgl = repp.tile([128, L], BF16, tag="sgl", bufs=2)
            nc.sync.dma_start(sgl, sg_d[g])
            yf[g] = xc[g]
            nc.vector.tensor_mul(yf[g], py, sgl)

        # ---------------- phase F: out_proj -> ypart ----------------------
        for tb in range(NTT):
            po = pp.tile([128, DM], F32, tag="ps")
            for g in range(NG):
                nc.tensor.matmul(po, yf[g][:, tb * 128:(tb + 1) * 128],
                                 w_out_sb[:, g, :],
                                 start=(g == 0), stop=(g == NG - 1))
            ot = lnp.tile([128, DM], BF16, tag="ot")
            nc.vector.tensor_copy(ot, po)
            nc.sync.dma_start(
                ypart[tb * 128:(tb + 1) * 128, :], ot)

        # ---------------- combine directions + final LayerNorm ------------
        nc.gpsimd.collective_compute(
            "AllGather", OP.bypass, replica_groups=REPLICA_GROUPS,
            ins=[ypart.opt()], outs=[gbuf.opt()],
        )
        for i in range(NTT):
            s0 = lnp.tile([128, DM], BF16, tag="s0")
            nc.sync.dma_start(s0, gbuf[0, i * 128:(i + 1) * 128, :])
            # direction-1 partial is in flipped time order: load the mirrored
            # block forward, then reverse rows via the exchange matrix
            s1 = lnp.tile([128, DM], BF16, tag="s1")
            nc.sync.dma_start(
                s1, gbuf[1, L - (i + 1) * 128: L - i * 128, :])
            xt2 = lnp.tile([128, DM], BF16, tag="xt2")
            nc.sync.dma_start(xt2, x_nat[i * 128:(i + 1) * 128, :])
            # sum all three contributions in PSUM via the tensor engine
            pj = pp.tile([128, DM], F32, tag="ps")
            nc.tensor.matmul(pj, eyej_sb, s1, start=True, stop=False)
            nc.tensor.matmul(pj, eyen_sb, s0, start=False, stop=False)
            nc.tensor.matmul(pj, eyen_sb, xt2, start=False, stop=True)
            s = lnp.tile([128, DM], F32, tag="ssum")
            nc.vector.tensor_copy(s, pj)
            st = lnp.tile([128, 6], F32, tag="st2")
            nc.vector.bn_stats(st, s)
            mv = lnp.tile([128, 2], F32, tag="mv2")
            nc.vector.bn_aggr(mv, st)
            nc.scalar.activation(mv[:, 1:2], mv[:, 1:2], AF.Ln,
                                 bias=eps_sb[:, 0:1])
            nc.scalar.activation(mv[:, 1:2], mv[:, 1:2], AF.Exp, scale=-0.5)
            nc.vector.tensor_scalar(out=s, in0=s, scalar1=mv[:, 0:1],
                                    scalar2=mv[:, 1:2],
                                    op0=OP.subtract, op1=OP.mult)
            nc.vector.tensor_mul(s, s, ln2w_sb)
            nc.vector.tensor_add(s, s, ln2b_sb)
            nc.sync.dma_start(out[i * 128:(i + 1) * 128, :], s)


# ---------------- host side ----------------

def make_core_inputs(inputs):
    """Build the 8 per-core input dicts from the full problem inputs."""
    x = np.ascontiguousarray(inputs["x"], dtype=np.float32)        # [B, L, DM]
    ln1_w = inputs["ln1_w"].astype(np.float32)
    ln1_b = inputs["ln1_b"].astype(np.float32)
    in_w = inputs["in_proj_w"].astype(np.float32)                  # [2, 2DI, DM]
    conv_w = inputs["conv_w"].astype(np.float32)                   # [2, DI, 4]
    conv_b = inputs["conv_b"].astype(np.float32)                   # [2, DI]
    xp_w = inputs["x_proj_w"].astype(np.float32)                   # [2, 64, DI]
    dt_w = inputs["dt_proj_w"].astype(np.float32)                  # [2, DI, DR]
    dt_b = inputs["dt_proj_b"].astype(np.float32)                  # [2, DI]
    a_log = inputs["A_log"].astype(np.float32)                     # [2, DI, DS]
    d_par = inputs["D_param"].astype(np.float32)                   # [2, DI]
    out_w = inputs["out_proj_w"].astype(np.float32)                # [2, DM, DI]
    ln2_w = inputs["ln2_w"].astype(np.float32)
    ln2_b = inputs["ln2_b"].astype(np.float32)

    bf = ml_dtypes.bfloat16
    eye = np.eye(128, dtype=np.float32)
    per_dir = []
    for d in range(2):
        w = in_w[d]                                   # [2DI, DM]
        w_in_T = np.ascontiguousarray((w * ln1_w[None, :]).T)      # [DM, 2DI]
        v = w @ ln1_b                                  # [2DI]
        csum = conv_w[d].sum(axis=1)                   # [DI]
        convb_adj = conv_b[d] + csum * v[:DI]
        silub_adj = v[DI:]
        convd = np.zeros((D_CONV, NG, 128, 128), np.float32)
        for k in range(D_CONV):
            for g in range(NG):
                np.fill_diagonal(convd[k, g], conv_w[d, g * 128:(g + 1) * 128, k])
        a_neg = -np.exp(a_log[d])                      # [DI, DS]
        per_dir.append(dict(
            w_in=w_in_T.astype(bf),
            convd=convd.astype(bf),
            convb=convb_adj.reshape(NG, 128),
            silub=silub_adj.reshape(NG, 128),
            w_xp=np.ascontiguousarray(xp_w[d].T).astype(bf),       # [DI, 64]
            w_dt=np.ascontiguousarray(dt_w[d].T).astype(bf),       # [DR, DI]
            dtb=dt_b[d].reshape(NG, 128),
            a_sc=np.ascontiguousarray(a_neg.reshape(NG, 128, DS)),
            d_vec=d_par[d].reshape(NG, 128),
            w_out=np.ascontiguousarray(out_w[d].T).astype(bf),     # [DI, DM]
        ))

    in_maps = []
    for core in range(N_CORES):
        d, b = core // 4, core % 4
        xb = x[b]
        m = dict(per_dir[d])
        m["x_d"] = xb if d == 0 else np.ascontiguousarray(xb[::-1])
        m["x_nat"] = xb.astype(bf)
        m["eye"] = eye
        m["eyen"] = eye.astype(bf)
        m["eyej"] = eye[::-1].copy().astype(bf)
        m["ln2w"] = ln2_w.reshape(1, DM)
        m["ln2b"] = ln2_b.reshape(1, DM)
        in_maps.append(m)
    return in_maps


_NC = None


def _get_module():
    global _NC
    if _NC is None:
        _NC = build_module()
    return _NC


def kernel(**inputs) -> np.ndarray:
    nc = _get_module()
    in_maps = make_core_inputs(inputs)
    res = run_bass_kernel_spmd(nc, in_maps, core_ids=list(range(N_CORES)))
    outs = [res.results[b]["out"] for b in range(B_SZ)]
    return np.stack(outs, axis=0)


if __name__ == "__main__":
    nc = build_module()
    print("module built ok")


# revision 8
# speedup vs baseline: 1.0692x; 1.0329x over previous
"""Bidirectional Mamba layer on 8 Trainium2 NeuronCores (Bass/Tile).

Sharding: 8 cores = 2 directions x 4 batch samples. Each core runs the full
Mamba block for its (direction, sample) pair; a pairwise AllGather combines
the two directions, and every core (redundantly, SPMD-symmetric) applies the
final residual + LayerNorm.

Layout: channels on partitions, time on the free axis, full L=2048 per op
(monolith — no time chunking, so the selective scan needs no state carry).
All matmuls run bf16 (single PE pass); elementwise work is bf16 for the DVE
2x mode. The per-(group, state) inner loop is:
  Act:  da = exp(dt * A[g,n])          (scalar engine)
  DVE/GPS: bt = p * B_n                (broadcast multiply)
  DVE:  h = tensor_tensor_scan(da, bt) (the irreducible 2 cyc/elem scan)
  DVE:  yn = h * C_n
  PE:   py += I @ yn                   (PSUM accumulation over states)
"""

import ml_dtypes
import numpy as np

import concourse.bass as bass
import concourse.bacc as bacc
import concourse.tile as tile
from concourse import mybir
from concourse.bass_utils import run_bass_kernel_spmd

# ---- problem shapes (hardcoded per contract) ----
B_SZ, L, DM = 4, 2048, 512
D_CONV, DS, DR = 4, 16, 32
DI = 1024                  # d_inner
E2 = 2 * DI                # in_proj rows
NG = DI // 128             # 8 channel blocks
NTT = L // 128             # 16 time tiles of 128
NSL = L // 512             # 4 psum-width slices
EPS = 1e-5
F32 = mybir.dt.float32
BF16 = mybir.dt.bfloat16
AF = mybir.ActivationFunctionType
OP = mybir.AluOpType

N_CORES = 8

# CoreSim does not implement Silu; tests flip this to use sigmoid+mult
USE_SILU = True
REPLICA_GROUPS = [[0, 4], [1, 5], [2, 6], [3, 7]]

# groups whose bt-multiply runs on GPSIMD instead of DVE
GPS_GS = (1, 5)


def _silu(nc, pool, out_tile, psum, bias=None):
    """out = silu(psum [+ bias]); Silu on HW, sigmoid+mult fallback for sim."""
    kw = {} if bias is None else {"bias": bias}
    if USE_SILU:
        nc.scalar.activation(out_tile, psum, AF.Silu, **kw)
    else:
        zb = pool.tile(list(out_tile.shape), F32, tag="_silu_zb", name="zb",
                       bufs=1)
        nc.scalar.activation(zb, psum, AF.Identity, **kw)
        nc.scalar.activation(out_tile, psum, AF.Sigmoid, **kw)
        nc.vector.tensor_mul(out_tile, zb, out_tile)


def build_module():
    nc = bacc.Bacc(
        "TRN2", target_bir_lowering=False, debug=False, num_devices=N_CORES
    )

    # ---------------- I/O ----------------
    x_d = nc.dram_tensor("x_d", [L, DM], F32, kind="ExternalInput")
    x_nat = nc.dram_tensor("x_nat", [L, DM], BF16, kind="ExternalInput")
    w_in = nc.dram_tensor("w_in", [DM, E2], BF16, kind="ExternalInput")
    convd = nc.dram_tensor("convd", [D_CONV, NG, 128, 128], BF16, kind="ExternalInput")
    convb = nc.dram_tensor("convb", [NG, 128], F32, kind="ExternalInput")
    silub = nc.dram_tensor("silub", [NG, 128], F32, kind="ExternalInput")
    w_xp = nc.dram_tensor("w_xp", [DI, DR + 2 * DS], BF16, kind="ExternalInput")
    w_dt = nc.dram_tensor("w_dt", [DR, DI], BF16, kind="ExternalInput")
    dtb = nc.dram_tensor("dtb", [NG, 128], F32, kind="ExternalInput")
    a_sc = nc.dram_tensor("a_sc", [NG, 128, DS], F32, kind="ExternalInput")
    d_vec = nc.dram_tensor("d_vec", [NG, 128], F32, kind="ExternalInput")
    w_out = nc.dram_tensor("w_out", [DI, DM], BF16, kind="ExternalInput")
    eye = nc.dram_tensor("eye", [128, 128], F32, kind="ExternalInput")
    eyen = nc.dram_tensor("eyen", [128, 128], BF16, kind="ExternalInput")
    eyej = nc.dram_tensor("eyej", [128, 128], BF16, kind="ExternalInput")
    ln2w = nc.dram_tensor("ln2w", [1, DM], F32, kind="ExternalInput")
    ln2b = nc.dram_tensor("ln2b", [1, DM], F32, kind="ExternalInput")
    out = nc.dram_tensor("out", [L, DM], F32, kind="ExternalOutput")

    with tile.TileContext(nc) as tc:
        build_program(
            tc, x_d, x_nat, w_in, convd, convb, silub, w_xp, w_dt, dtb,
            a_sc, d_vec, w_out, eye, eyen, eyej, ln2w, ln2b, out,
        )
    nc.compile()
    return nc


def build_program(tc, x_d, x_nat, w_in, convd, convb, silub, w_xp, w_dt, dtb,
                  a_sc, d_vec, w_out, eye, eyen, eyej, ln2w, ln2b, out):
    nc = tc.nc
    from contextlib import ExitStack

    ctx = ExitStack()
    with ctx:
        wp = ctx.enter_context(tc.tile_pool(name="wp", bufs=1))
        lnp = ctx.enter_context(tc.tile_pool(name="lnp", bufs=2))
        big = ctx.enter_context(tc.tile_pool(name="big", bufs=1))
        snp = ctx.enter_context(tc.tile_pool(name="snp", bufs=3))
        repp = ctx.enter_context(tc.tile_pool(name="repp", bufs=3))
        dramp = ctx.enter_context(tc.tile_pool(name="dramp", bufs=2, space="DRAM"))
        pp_ctx = tc.tile_pool(name="pp", space="PSUM", bufs=2)
        pp = pp_ctx.__enter__()

        # ---------------- load weights ----------------
        w_in_sb = wp.tile([128, DM // 128, E2], BF16)
        for k in range(DM // 128):
            nc.sync.dma_start(w_in_sb[:, k, :], w_in[k * 128:(k + 1) * 128, :])
        convd_sb = wp.tile([128, D_CONV, NG, 128], BF16)
        for k in range(D_CONV):
            for g in range(NG):
                nc.sync.dma_start(convd_sb[:, k, g, :], convd[k, g])
        w_xp_sb = wp.tile([128, NG, DR + 2 * DS], BF16)
        for g in range(NG):
            nc.sync.dma_start(w_xp_sb[:, g, :], w_xp[g * 128:(g + 1) * 128, :])
        w_dt_sb = wp.tile([DR, DI], BF16)
        nc.sync.dma_start(w_dt_sb, w_dt[:, :])
        w_out_sb = wp.tile([128, NG, DM], BF16)
        for g in range(NG):
            nc.sync.dma_start(w_out_sb[:, g, :], w_out[g * 128:(g + 1) * 128, :])
        a_sb = wp.tile([128, NG, DS], F32)
        for g in range(NG):
            nc.sync.dma_start(a_sb[:, g, :], a_sc[g])
        eye_sb = wp.tile([128, 128], F32)
        nc.sync.dma_start(eye_sb, eye[:, :])
        eyen_sb = wp.tile([128, 128], BF16)
        nc.sync.dma_start(eyen_sb, eyen[:, :])
        eyej_sb = wp.tile([128, 128], BF16)
        nc.sync.dma_start(eyej_sb, eyej[:, :])

        def col_load(name, src):
            t = wp.tile([128, NG], F32, name=name)
            for g in range(NG):
                nc.sync.dma_start(t[:, g:g + 1], src[g:g + 1, :].rearrange("a b -> b a"))
            return t

        convb_sb = col_load("convb_sb", convb)
        silub_sb = col_load("silub_sb", silub)
        dtb_sb = col_load("dtb_sb", dtb)
        d_sb = col_load("d_sb", d_vec)

        ln2w_sb = wp.tile([128, DM], F32)
        nc.sync.dma_start(ln2w_sb, ln2w[0:1, :].to_broadcast([128, DM]))
        ln2b_sb = wp.tile([128, DM], F32)
        nc.sync.dma_start(ln2b_sb, ln2b[0:1, :].to_broadcast([128, DM]))

        eps_sb = wp.tile([128, 1], F32)
        nc.vector.memset(eps_sb, EPS)

        ypart = dramp.tile([L, DM], BF16, bufs=1)
        sg_d = dramp.tile([NG, 128, L], BF16, bufs=1)
        gbuf = dramp.tile([2, L, DM], BF16, bufs=1)
        bc_d = dramp.tile([2 * DS, L], BF16, bufs=1)

        # ---------------- phase A: LayerNorm1 + transpose ----------------
        # xn_t: [dm-part, k-block, t] bf16, consumed by in_proj
        xn_t = big.tile([128, DM // 128, L], BF16, name="xn_t")
        for i in range(NTT):
            xt = lnp.tile([128, DM], F32, tag="xt")
            nc.sync.dma_start(xt, x_d[i * 128:(i + 1) * 128, :])
            st = lnp.tile([128, 6], F32, tag="st")
            nc.vector.bn_stats(st, xt)
            mv = lnp.tile([128, 2], F32, tag="mv")
            nc.vector.bn_aggr(mv, st)
            nc.scalar.activation(mv[:, 1:2], mv[:, 1:2], AF.Ln,
                                 bias=eps_sb[:, 0:1])
            nc.scalar.activation(mv[:, 1:2], mv[:, 1:2], AF.Exp, scale=-0.5)
            nc.vector.tensor_scalar(out=xt, in0=xt, scalar1=mv[:, 0:1],
                                    scalar2=mv[:, 1:2],
                                    op0=OP.subtract, op1=OP.mult)
            for kb in range(DM // 128):
                ps_t = pp.tile([128, 512], F32, tag="ps")
                nc.tensor.transpose(ps_t[:, 0:128], xt[:, kb * 128:(kb + 1) * 128],
                                    eye_sb)
                nc.vector.tensor_copy(xn_t[:, kb, i * 128:(i + 1) * 128],
                                      ps_t[:, 0:128])

        # ---------------- phase B: in_proj ----------------
        # xc half into conv staging (3-col zero halo at front)
        xzf = [big.tile([128, L + 3], BF16, name=f"xzf{g}") for g in range(NG)]
        for g in range(NG):
            nc.vector.memset(xzf[g][:, 0:3], 0.0)
            for s in range(NSL):
                pz = pp.tile([128, 512], F32, tag="ps")
                for k in range(DM // 128):
                    nc.tensor.matmul(pz, w_in_sb[:, k, g * 128:(g + 1) * 128],
                                     xn_t[:, k, s * 512:(s + 1) * 512],
                                     start=(k == 0), stop=(k == DM // 128 - 1))
                nc.vector.tensor_copy(xzf[g][:, 3 + s * 512:3 + (s + 1) * 512], pz)

        # z half + SiLU gate -> sg (spilled to DRAM, reloaded at gate)
        for g in range(NG):
            for s in range(NSL):
                pz = pp.tile([128, 512], F32, tag="ps")
                for k in range(DM // 128):
                    nc.tensor.matmul(
                        pz, w_in_sb[:, k, DI + g * 128: DI + (g + 1) * 128],
                        xn_t[:, k, s * 512:(s + 1) * 512],
                        start=(k == 0), stop=(k == DM // 128 - 1))
                sgt = snp.tile([128, 512], BF16, tag="sgt", bufs=2)
                _silu(nc, snp, sgt, pz, silub_sb[:, g:g + 1])
                nc.sync.dma_start(sg_d[g][:, s * 512:(s + 1) * 512], sgt)

        # ---------------- phase C: depthwise causal conv + SiLU ----------
        xc = [big.tile([128, L], BF16, name=f"xc{g}") for g in range(NG)]
        for g in range(NG):
            for s in range(NSL):
                pc = pp.tile([128, 512], F32, tag="ps")
                for k in range(D_CONV):
                    nc.tensor.matmul(pc, convd_sb[:, k, g, :],
                                     xzf[g][:, k + s * 512: k + (s + 1) * 512],
                                     start=(k == 0), stop=(k == D_CONV - 1))
                _silu(nc, snp, xc[g][:, s * 512:(s + 1) * 512], pc,
                      convb_sb[:, g:g + 1])

        # ---------------- phase D: x_proj -> (dt_r, B, C) -----------------
        xdbl = big.tile([64, L], BF16, name="xdbl")
        for s in range(NSL):
            pxd = pp.tile([64, 512], F32, tag="pxd", bufs=1)
            for g in range(NG):
                nc.tensor.matmul(pxd, w_xp_sb[:, g, :],
                                 xc[g][:, s * 512:(s + 1) * 512],
                                 start=(g == 0), stop=(g == NG - 1))
            nc.vector.tensor_copy(xdbl[:, s * 512:(s + 1) * 512], pxd)
        nc.sync.dma_start(bc_d, xdbl[DR:DR + 2 * DS, :])

        # dt = softplus(dt_proj @ dt_r + bias); p = dt * xc
        # dt_c[0..3] reuse xn_t's space, dt_c[4..7] reuse w_in_sb (both dead
        # after in_proj); p_c reuses the conv staging xzf (dead after conv)
        dt_c = [xn_t[:, g, :] if g < 4 else w_in_sb[:, g - 4, :]
                for g in range(NG)]
        p_c = [xzf[g][:, 0:L] for g in range(NG)]
        for g in range(NG):
            for s in range(NSL):
                sl = slice(s * 512, (s + 1) * 512)
                pdt = pp.tile([128, 512], F32, tag="ps")
                nc.tensor.matmul(pdt, w_dt_sb[:, g * 128:(g + 1) * 128],
                                 xdbl[0:DR, sl],
                                 start=True, stop=True)
                nc.scalar.activation(dt_c[g][:, sl], pdt, AF.Exp,
                                     bias=dtb_sb[:, g:g + 1])
                nc.scalar.activation(dt_c[g][:, sl], dt_c[g][:, sl],
                                     AF.Ln, bias=1.0)
            nc.vector.tensor_mul(p_c[g], dt_c[g], xc[g])

        # ---------------- phase E: selective scan ------------------------
        # wave-outer (4 groups share each B/C broadcast), time in halves of
        # 1024 so four 2-bank PSUM accumulators fit; h carried across halves
        pp_ctx.__exit__(None, None, None)
        HW = L // 2
        yf = [xc[g] for g in range(NG)]
        with tc.tile_pool(name="ppy", space="PSUM", bufs=1) as ppy:
            for w0 in (0, 4):
                wave = range(w0, w0 + 4)
                hcar = snp.tile([128, 4, DS], BF16, tag="hcar", bufs=2,
                                name=f"hcar{w0}")
                for half in (0, 1):
                    hs = slice(half * HW, (half + 1) * HW)
                    py = [ppy.tile([128, HW], F32, tag=f"py{j}", bufs=1,
                                   name=f"py{j}") for j in range(4)]
                    for j, g in enumerate(wave):
                        yd = snp.tile([128, HW], BF16, tag="yd", bufs=2)
                        nc.vector.tensor_scalar_mul(yd, xc[g][:, hs],
                                                    d_sb[:, g:g + 1])
                        for s in range(2):
                            nc.tensor.matmul(py[j][:, s * 512:(s + 1) * 512],
                                             eyen_sb,
                                             yd[:, s * 512:(s + 1) * 512],
                                             start=True, stop=False)
                    for n in range(DS):
                        brep = repp.tile([128, HW], BF16, tag="brep", bufs=3)
                        nc.sync.dma_start(
                            brep, bc_d[n:n + 1, hs].to_broadcast([128, HW]))
                        crep = repp.tile([128, HW], BF16, tag="crep", bufs=3)
                        nc.sync.dma_start(
                            crep,
                            bc_d[DS + n:DS + n + 1, hs].to_broadcast([128, HW]))
                        for j, g in enumerate(wave):
                            da = snp.tile([128, HW], BF16, tag="da", bufs=3)
                            nc.scalar.activation(da, dt_c[g][:, hs], AF.Exp,
                                                 scale=a_sb[:, g, n:n + 1])
                            if g in GPS_GS:
                                bt = snp.tile([128, HW], BF16, tag="bt",
                                              name="btg", bufs=3)
                                nc.gpsimd.tensor_tensor(bt, p_c[g][:, hs],
                                                        brep, op=OP.mult)
                            else:
                                bt = snp.tile([128, HW], BF16, tag="bt",
                                              name="btd", bufs=3)
                                nc.vector.tensor_mul(bt, p_c[g][:, hs], brep)
                            h = snp.tile([128, HW], BF16, tag="h", bufs=2)
                            init = 0.0 if half == 0 else hcar[:, j, n:n + 1]
                            nc.vector.tensor_tensor_scan(h, da, bt,
                                                         initial=init,
                                                         op0=OP.mult,
                                                         op1=OP.add)
                            if half == 0:
                                nc.vector.tensor_copy(hcar[:, j, n:n + 1],
                                                      h[:, HW - 1:HW])
                            yn = snp.tile([128, HW], BF16, tag="yn", bufs=2)
                            nc.vector.tensor_mul(yn, h, crep)
                            for s in range(2):
                                nc.tensor.matmul(
                                    py[j][:, s * 512:(s + 1) * 512], eyen_sb,
                                    yn[:, s * 512:(s + 1) * 512],
                                    start=False, stop=(n == DS - 1))
                    # gate with silu(z); write into xc[g]'s tile (xc dead now)
                    for j, g in enumerate(wave):
                        sgl = repp.tile([128, HW], BF16, tag="sgl", bufs=2)
                        nc.sync.dma_start(sgl, sg_d[g][:, hs])
                        nc.vector.tensor_mul(yf[g][:, hs], py[j], sgl)

        # ---------------- phase F: out_proj -> ypart ----------------------
        pp2_ctx = tc.tile_pool(name="pp2", space="PSUM", bufs=2)
        pp = pp2_ctx.__enter__()
        for tb in range(NTT):
            po = pp.tile([128, DM], F32, tag="ps")
            for g in range(NG):
                nc.tensor.matmul(po, yf[g][:, tb * 128:(tb + 1) * 128],
                                 w_out_sb[:, g, :],
                                 start=(g == 0), stop=(g == NG - 1))
            ot = lnp.tile([128, DM], BF16, tag="ot")
            nc.vector.tensor_copy(ot, po)
            nc.sync.dma_start(
                ypart[tb * 128:(tb + 1) * 128, :], ot)

        # ---------------- combine directions + final LayerNorm ------------
        nc.gpsimd.collective_compute(
            "AllGather", OP.bypass, replica_groups=REPLICA_GROUPS,
            ins=[ypart.opt()], outs=[gbuf.opt()],
        )
        for i in range(NTT):
            s0 = lnp.tile([128, DM], BF16, tag="s0")
            nc.sync.dma_start(s0, gbuf[0, i * 128:(i + 1) * 128, :])
            # direction-1 partial is in flipped time order: load the mirrored
            # block forward, then reverse rows via the exchange matrix
            s1 = lnp.tile([128, DM], BF16, tag="s1")
            nc.sync.dma_start(
                s1, gbuf[1, L - (i + 1) * 128: L - i * 128, :])
            xt2 = lnp.tile([128, DM], BF16, tag="xt2")
            nc.sync.dma_start(xt2, x_nat[i * 128:(i + 1) * 128, :])
            # sum all three contributions in PSUM via the tensor engine
            pj = pp.tile([128, DM], F32, tag="ps")
            nc.tensor.matmul(pj, eyej_sb, s1, start=True, stop=False)
            nc.tensor.matmul(pj, eyen_sb, s0, start=False, stop=False)
            nc.tensor.matmul(pj, eyen_sb, xt2, start=False, stop=True)
            s = lnp.tile([128, DM], F32, tag="ssum")
            nc.vector.tensor_copy(s, pj)
            st = lnp.tile([128, 6], F32, tag="st2")
            nc.vector.bn_stats(st, s)
            mv = lnp.tile([128, 2], F32, tag="mv2")
            nc.vector.bn_aggr(mv, st)
            nc.scalar.activation(mv[:, 1:2], mv[:, 1:2], AF.Ln,
                                 bias=eps_sb[:, 0:1])
            nc.scalar.activation(mv[:, 1:2], mv[:, 1:2], AF.Exp, scale=-0.5)
            nc.vector.tensor_scalar(out=s, in0=s, scalar1=mv[:, 0:1],
                                    scalar2=mv[:, 1:2],
                                    op0=OP.subtract, op1=OP.mult)
            nc.vector.tensor_mul(s, s, ln2w_sb)
            nc.vector.tensor_add(s, s, ln2b_sb)
            nc.sync.dma_start(out[i * 128:(i + 1) * 128, :], s)
        pp2_ctx.__exit__(None, None, None)


# ---------------- host side ----------------

def make_core_inputs(inputs):
    """Build the 8 per-core input dicts from the full problem inputs."""
    x = np.ascontiguousarray(inputs["x"], dtype=np.float32)        # [B, L, DM]
    ln1_w = inputs["ln1_w"].astype(np.float32)
    ln1_b = inputs["ln1_b"].astype(np.float32)
    in_w = inputs["in_proj_w"].astype(np.float32)                  # [2, 2DI, DM]
    conv_w = inputs["conv_w"].astype(np.float32)                   # [2, DI, 4]
    conv_b = inputs["conv_b"].astype(np.float32)                   # [2, DI]
    xp_w = inputs["x_proj_w"].astype(np.float32)                   # [2, 64, DI]
    dt_w = inputs["dt_proj_w"].astype(np.float32)                  # [2, DI, DR]
    dt_b = inputs["dt_proj_b"].astype(np.float32)                  # [2, DI]
    a_log = inputs["A_log"].astype(np.float32)                     # [2, DI, DS]
    d_par = inputs["D_param"].astype(np.float32)                   # [2, DI]
    out_w = inputs["out_proj_w"].astype(np.float32)                # [2, DM, DI]
    ln2_w = inputs["ln2_w"].astype(np.float32)
    ln2_b = inputs["ln2_b"].astype(np.float32)

    bf = ml_dtypes.bfloat16
    eye = np.eye(128, dtype=np.float32)
    per_dir = []
    for d in range(2):
        w = in_w[d]                                   # [2DI, DM]
        w_in_T = np.ascontiguousarray((w * ln1_w[None, :]).T)      # [DM, 2DI]
        v = w @ ln1_b                                  # [2DI]
        csum = conv_w[d].sum(axis=1)                   # [DI]
        convb_adj = conv_b[d] + csum * v[:DI]
        silub_adj = v[DI:]
        convd = np.zeros((D_CONV, NG, 128, 128), np.float32)
        for k in range(D_CONV):
            for g in range(NG):
                np.fill_diagonal(convd[k, g], conv_w[d, g * 128:(g + 1) * 128, k])
        a_neg = -np.exp(a_log[d])                      # [DI, DS]
        per_dir.append(dict(
            w_in=w_in_T.astype(bf),
            convd=convd.astype(bf),
            convb=convb_adj.reshape(NG, 128),
            silub=silub_adj.reshape(NG, 128),
            w_xp=np.ascontiguousarray(xp_w[d].T).astype(bf),       # [DI, 64]
            w_dt=np.ascontiguousarray(dt_w[d].T).astype(bf),       # [DR, DI]
            dtb=dt_b[d].reshape(NG, 128),
            a_sc=np.ascontiguousarray(a_neg.reshape(NG, 128, DS)),
            d_vec=d_par[d].reshape(NG, 128),
            w_out=np.ascontiguousarray(out_w[d].T).astype(bf),     # [DI, DM]
        ))

    in_maps = []
    for core in range(N_CORES):
        d, b = core // 4, core % 4
        xb = x[b]
        m = dict(per_dir[d])
        m["x_d"] = xb if d == 0 else np.ascontiguousarray(xb[::-1])
        m["x_nat"] = xb.astype(bf)
        m["eye"] = eye
        m["eyen"] = eye.astype(bf)
        m["eyej"] = eye[::-1].copy().astype(bf)
        m["ln2w"] = ln2_w.reshape(1, DM)
        m["ln2b"] = ln2_b.reshape(1, DM)
        in_maps.append(m)
    return in_maps


_NC = None


def _get_module():
    global _NC
    if _NC is None:
        _NC = build_module()
    return _NC


def kernel(**inputs) -> np.ndarray:
    nc = _get_module()
    in_maps = make_core_inputs(inputs)
    res = run_bass_kernel_spmd(nc, in_maps, core_ids=list(range(N_CORES)))
    outs = [res.results[b]["out"] for b in range(B_SZ)]
    return np.stack(outs, axis=0)


if __name__ == "__main__":
    nc = build_module()
    print("module built ok")
